# revision 17
# baseline (speedup 1.0000x reference)
"""Trainium2 Bass kernel for nn_CrossFrameAttention (sparse_attention).

Reference math per batch b:
    attn  = softmax_over_SHW(q @ K) + mask          (mask is per-key, query-independent)
    out   = attn @ V
which decomposes into  softmax(qK)V  +  (mask @ V)  where the second term is a
rank-1, query-independent bias handled on host.

Device strategy (8 NeuronCores): batch (2) x key-shard (4). Scores are computed
TRANSPOSED (keys on PSUM partitions, queries on the free axis) so that:
  - QK needs no transposes and the AV matmul consumes exp(scores) directly
  - softmax denominators come for free from a ones-column appended to V

This version is tuned around the scalar (ACT) engine, which is the hard
bottleneck: exp of 8192x4096 scores per core at 1 elem/cycle/lane/1.2GHz.
  - exp runs as [128, 1536] instructions (3 PSUM banks per score slot, 2 slots
    + 2 AV-accumulator banks = all 8 banks), amortizing the ~350-cycle
    per-instruction overhead better than the 2-bank slots of the baseline.
  - The numerical-stability shift is applied INSIDE the exp via the ACT bias
    port (per-partition broadcast) instead of a 65th contraction row. That
    keeps the QK contraction at exactly 64, which lets pairs of key tiles run
    as CONCURRENT row-tiled matmuls (tile_position row bands 0:64 / 64:128,
    stacked keys + duplicated queries), roughly halving QK time on hardware
    and guaranteeing the PE hides fully under the ACT engine.
  - The bias is per query CHUNK (512 queries): queries are permuted on host in
    ascending order of an upper bound mhat(q) on their max score, and each
    chunk uses shift = max(mhat in chunk) - 70. The bound is
    max(exact max over the 1024 largest-norm keys, ||q|| * ||k||_{1025th}),
    cheap on host and tight enough that every chunk's softmax stays well
    inside fp32 range (validated: denominators within [1e-31, 3e30]).
QK operands are float32r (fp32 storage, 12-mantissa-bit PE inputs, exact fp32
accumulation at the bf16 streaming rate). The AV matmul uses bf16 V and P:
fp32r matmuls self-load their stationary operand, and a same-row-band weight
load cannot overlap the preceding matmul (measured ~445 vs 214 ns/MM) — bf16
weights go through the background weight buffer, restoring full streaming
rate for the value matmuls. exp(score) quantization to bf16 largely cancels
between numerator and the ones-column denominator.
"""

import ml_dtypes
import numpy as np

import concourse.bacc as bacc
import concourse.mybir as mybir
import concourse.tile as tile
from concourse.bass_utils import run_bass_kernel_spmd

S, B, CK, CV, H, W = 8, 2, 64, 64, 64, 64
HW, SHW = H * W, S * H * W
N_CORES = 8
KEY_SHARDS = 4                 # key-parallel cores per batch
KC = SHW // KEY_SHARDS         # 8192 keys per core
NKT = KC // 128                # 64 key tiles of 128 keys
QCH = 512                      # queries per chunk (= one PSUM bank of fp32)
NQC = HW // QCH                # 8 query chunks
SLOT = 3                       # key tiles (PSUM banks) per exp instruction
RELAX = 70.0                   # shift relaxation: p <= e^70, sum-p <= 2e34
TOPK = 1024                    # keys given an exact host-side max for the bound
RADIUS, WEIGHT = 0.1, 0.2

F32 = mybir.dt.float32
BF16 = mybir.dt.bfloat16
F32R = mybir.dt.float32r  # fp32 storage; PE truncates inputs to 12 mantissa
                          # bits and accumulates exactly, at bf16 speed

_compiled_nc = None


def _kernel_body(tc, keys, qry, vals, bias, out, repeat=1):
    nc = tc.nc
    with (
        tc.tile_pool(name="persist", bufs=1) as persist,
        tc.tile_pool(name="p_pool", bufs=3) as p_pool,
        tc.tile_pool(name="o_pool", bufs=2) as o_pool,
        tc.tile_pool(name="ps_sc", bufs=2, space="PSUM") as ps_sc,
        tc.tile_pool(name="ps_out", bufs=2, space="PSUM") as ps_out,
    ):
        # keys row-stacked: col-block pb holds key tile 2*pb on partitions
        # 0:64 and tile 2*pb+1 on partitions 64:128 (64 dims each)
        keys_sb = persist.tile([128, (NKT // 2) * 128], F32R)
        q_sb = persist.tile([128, HW], F32R)        # q duplicated on both halves
        vals_sb = persist.tile([128, NKT * (CV + 1)], BF16)
        bias_sb = persist.tile([128, NQC], F32)     # -shift per query chunk

        def chunks(total, sizes):
            off = 0
            for s in sizes:
                yield off, min(s, total - off)
                off += s
                if off >= total:
                    break

        key_chunks = list(chunks((NKT // 2) * 128, [256, 768, 1024, 2048]))
        q_chunks = list(chunks(HW, [512, 1024, 2560]))
        val_chunks = list(chunks(NKT * (CV + 1), [260, 1040, 2860]))
        dmas = [
            (bias_sb, bias, (0, NQC)),
            (q_sb, qry, q_chunks[0]),
            (keys_sb, keys, key_chunks[0]),
            (vals_sb, vals, val_chunks[0]),
            (keys_sb, keys, key_chunks[1]),
            (vals_sb, vals, val_chunks[1]),
            (q_sb, qry, q_chunks[1]),
            (keys_sb, keys, key_chunks[2]),
            (vals_sb, vals, val_chunks[2]),
            (q_sb, qry, q_chunks[2]),
            (keys_sb, keys, key_chunks[3]),
        ]
        for sb, dram, (off, w) in dmas:
            nc.sync.dma_start(out=sb[:, off:off + w], in_=dram[:, off:off + w])

        # software-pipelined emission: each slot's AV matmuls are emitted AFTER
        # the next slot's QK matmuls, so the PE's in-order queue always holds
        # ready work (next QK) while the current exp runs — the ACT engine
        # (the bottleneck) then never waits on the PE.
        pending = None  # (p_tile, n, e, chunk, out_ps)

        def emit_av(p, n, e, c, out_ps):
            for j in range(n):
                t = e + j
                nc.tensor.matmul(
                    out=out_ps,
                    lhsT=vals_sb[:, t * (CV + 1):(t + 1) * (CV + 1)],
                    rhs=p[:, j * QCH:(j + 1) * QCH],
                    start=(t == 0),
                    stop=(t == NKT - 1),
                    skip_group_check=True,
                )
            if e + n == NKT:
                o_sb = o_pool.tile([CV + 1, QCH], F32)
                nc.vector.tensor_copy(out=o_sb, in_=out_ps)
                nc.sync.dma_start(out=out[:, c * QCH:(c + 1) * QCH], in_=o_sb)

        for rep in range(repeat):
            for c in range(NQC):
                out_ps = ps_out.tile([CV + 1, QCH], F32)
                e = 0
                while e < NKT:
                    n = min(SLOT, NKT - e)
                    sc = ps_sc.tile([128, SLOT * QCH], F32, tag="sc")
                    for j in range(n):
                        t = e + j
                        pb, mem = divmod(t, 2)
                        rows = slice(64 * mem, 64 * (mem + 1))
                        nc.tensor.matmul(
                            out=sc[:, j * QCH:(j + 1) * QCH],
                            lhsT=keys_sb[rows, pb * 128:(pb + 1) * 128],
                            rhs=q_sb[rows, c * QCH:(c + 1) * QCH],
                            start=True,
                            stop=True,
                        )
                    p = p_pool.tile([128, SLOT * QCH], BF16, tag="p")
                    nc.scalar.activation(
                        out=p[:, :n * QCH], in_=sc[:, :n * QCH],
                        func=mybir.ActivationFunctionType.Exp,
                        bias=bias_sb[:, c:c + 1],
                    )
                    if pending is not None:
                        emit_av(*pending)
                    pending = (p, n, e, c, out_ps)
                    e += n
        if pending is not None:
            emit_av(*pending)


def _build(repeat=1):
    nc = bacc.Bacc("TRN2", target_bir_lowering=False, debug=False, num_devices=N_CORES)
    keys = nc.dram_tensor("keys", [128, (NKT // 2) * 128], F32R, kind="ExternalInput").ap()
    qry = nc.dram_tensor("qry", [128, HW], F32R, kind="ExternalInput").ap()
    vals = nc.dram_tensor("vals", [128, NKT * (CV + 1)], BF16, kind="ExternalInput").ap()
    bias = nc.dram_tensor("bias", [128, NQC], F32, kind="ExternalInput").ap()
    out = nc.dram_tensor("out", [CV + 1, HW], F32, kind="ExternalOutput").ap()
    with tile.TileContext(nc) as tc:
        _kernel_body(tc, keys, qry, vals, bias, out, repeat=repeat)
    nc.compile()
    return nc


def _get_compiled():
    global _compiled_nc
    if _compiled_nc is None:
        _compiled_nc = _build()
    return _compiled_nc


def _prep_inputs(mk, mv, qq):
    """Build the 8 per-core input dicts from the full fp32 arrays.

    Returns (in_maps, vals_f, perms): perms[b] is the query permutation
    applied on device for batch b (output must be scattered back).
    """
    keys_f = mk.transpose(1, 2, 0, 3, 4).reshape(B, CK, SHW)     # [B, 64, 32768]
    vals_f = mv.transpose(1, 0, 3, 4, 2).reshape(B, SHW, CV)     # [B, 32768, 64]
    q_f = qq.reshape(B, CK, HW)                                  # [B, 64, 4096]

    perms, q_stacks, biases = [], [], []
    for b in range(B):
        qn = np.linalg.norm(q_f[b].astype(np.float64), axis=0)
        kn = np.linalg.norm(keys_f[b].astype(np.float64), axis=0)
        top = np.argpartition(kn, -TOPK)[-TOPK:]
        rest_max = np.partition(kn, -TOPK)[:-TOPK].max()
        # upper bound on each query's max score: exact over the top-norm
        # keys, Cauchy-Schwarz over the rest
        mt = (q_f[b].T.astype(np.float64) @ keys_f[b][:, top].astype(np.float64)).max(1)
        mhat = np.maximum(mt, qn * rest_max)
        perm = np.argsort(mhat)
        shifts = mhat[perm].reshape(NQC, QCH).max(1) - RELAX     # [NQC]
        perms.append(perm)
        q_stacks.append(
            np.ascontiguousarray(
                np.concatenate([q_f[b][:, perm]] * 2, axis=0), dtype=np.float32
            )
        )
        biases.append(
            np.ascontiguousarray(
                np.broadcast_to(-shifts.astype(np.float32), (128, NQC))
            )
        )

    in_maps = []
    for c in range(N_CORES):
        b, j = divmod(c, KEY_SHARDS)
        ksl = keys_f[b][:, j * KC:(j + 1) * KC]                   # [64, 8192]
        k3 = ksl.reshape(CK, NKT // 2, 2, 128)
        keys_st = np.concatenate(
            [k3[:, :, 0, :].reshape(CK, -1), k3[:, :, 1, :].reshape(CK, -1)],
            axis=0,
        )                                                         # [128, 4096]
        va = np.concatenate(
            [vals_f[b][j * KC:(j + 1) * KC], np.ones((KC, 1), np.float32)], axis=1
        )                                                         # [8192, 65]
        vals_re = va.reshape(NKT, 128, CV + 1).transpose(1, 0, 2).reshape(128, -1)
        vals_re = vals_re.astype(ml_dtypes.bfloat16)
        in_maps.append(
            {
                "keys": np.ascontiguousarray(keys_st, dtype=np.float32),
                "qry": q_stacks[b],
                "vals": np.ascontiguousarray(vals_re),
                "bias": biases[b],
            }
        )
    return in_maps, vals_f, perms


def kernel(memory_keys, memory_values, query_query, disparity, sequence_index):
    mk = np.asarray(memory_keys, dtype=np.float32)
    mv = np.asarray(memory_values, dtype=np.float32)
    qq = np.asarray(query_query, dtype=np.float32)
    dsp = np.asarray(disparity, dtype=np.float32)
    sqi = np.asarray(sequence_index)

    in_maps, vals_f, perms = _prep_inputs(mk, mv, qq)
    nc = _get_compiled()
    res = run_bass_kernel_spmd(nc, in_maps, list(range(N_CORES))).results

    # host epilogue: combine shards, normalize, unpermute, add rank-1 mask bias
    idx = sqi.astype(np.float32)
    dist = np.sqrt((idx[:, :, 1] - 5.0) ** 2 + (idx[:, :, 0] - 5.0) ** 2)   # [B, S]
    total_disp = dist[:, :, None, None] * dsp                               # [B, S, H, W]
    weight = WEIGHT / S / H / W
    mask = np.where(np.abs(total_disp) > RADIUS, weight, 0.0).reshape(B, SHW)
    bias = np.einsum("bm,bmv->bv", mask.astype(np.float64), vals_f.astype(np.float64))

    out = np.empty((B, CV, H, W), np.float32)
    for b in range(B):
        acc = np.zeros((CV + 1, HW), np.float64)
        for j in range(KEY_SHARDS):
            acc += res[b * KEY_SHARDS + j]["out"]
        o = acc[:CV] / acc[CV]
        unperm = np.empty_like(o)
        unperm[:, perms[b]] = o
        out[b] = (unperm + bias[b][:, None]).astype(np.float32).reshape(CV, H, W)
    return out
